# revision 4
# baseline (speedup 1.0000x reference)
"""Trainium2 Bass kernel for nn_Attention: per-head QKV attention + out-proj.

Contract: kernel(**inputs) takes FULL unsharded inputs
  x [8, 1024, 768] f32, Wqkv [12, 768, 192] f32, bqkv [12, 192] f32,
  Wo [768, 768] f32, bo [768] f32
returns FULL output [8, 1024, 768] f32.

Strategy: pure data-parallel over batch (8 batches -> 8 NeuronCores), no
collectives.  Each core computes its batch end-to-end in bf16 matmuls.

Math notes:
  - softmax rows sum to 1 => attn @ (v + bv) = attn @ v + bv, and since the
    attention output is immediately projected, bv folds into the projection
    bias: bo2 = bo + concat(bv) @ Wo.  V-bias never touches the device.
  - x is transposed + bf16-cast on HOST (xT [768, 1024]) -- kills the 48 PE
    transposes, the f32 x DMA, and the startup serialization.
  - softmax is computed unnormalized; the denominator r[q] = sum_k et[k, q]
    is built by accumulating the bf16 exp chunks on DVE (S += et_chunk) and
    one gpsimd.partition_all_reduce per head-pair, which also broadcasts r
    to all partitions (no DRAM bounce, no PE ones-column).  This frees the
    PV stationary to hold exactly 64 v-columns per head, so the two heads of
    a pair run as CONCURRENT col-tiled matmuls (array cols 0-63 / 64-127).
  - output is written bf16 and upcast on host (halves the out DMA).

Schedule: flat software pipeline over 48 (pair, sk) chunks; pv(j-2) rides 2
chunk-slots behind scores(j); v-projection chunks fill pair 0; q/k
projections of pair t+1 injected mid-pair.  All PE instructions chained
with no-sync ordering edges so the Tile scheduler preserves the interleave.
"""

import math
import os

import numpy as np
import ml_dtypes

import concourse.bass as bass
import concourse.tile as tile
from concourse import bacc, mybir
from concourse.bass_isa import ReduceOp
from concourse.bass_utils import run_bass_kernel_spmd
from concourse.tile_rust import add_dep_helper

B, S, D, H, HD = 8, 1024, 768, 12, 64
SCALE = 1.0 / math.sqrt(D)
FP = mybir.dt.float32
BF = mybir.dt.bfloat16
KC = D // 128   # 6 contraction chunks
SC = S // 128   # 8 seq chunks
NQ = S // 512   # 2 free-dim chunks of 512
NP = H // 2     # 6 head pairs

AluOp = mybir.AluOpType
ActFn = mybir.ActivationFunctionType

# Results of the last hardware run (for test harness introspection).
last_results = None


def _build_kernel_body(tc, out_d, xT_d, wqk_d, wv_d, wo_d, bqk_d, bo2_d):
    nc = tc.nc

    # Chain every TensorE instruction to the previous one with a no-sync
    # ordering edge: the Tile scheduler otherwise reorders the PE stream by
    # modeled readiness, undoing the deliberate scores/PV/QKV interleave.
    _pe_last = [None]

    def MM(*a, reuse_w=False, **k):
        inst = nc.tensor.matmul(*a, **k)
        if reuse_w:
            # stationary operand identical to the previous matmul in the
            # chained PE stream: skip the redundant LDWEIGHTS (bf16-safe)
            inst.ins.ldweights = False
        if _pe_last[0] is not None:
            add_dep_helper(inst.ins, _pe_last[0].ins, sync=False,
                           reason="pe-order")
        _pe_last[0] = inst
        return inst

    from contextlib import ExitStack

    with ExitStack() as ctx:
        wpool = ctx.enter_context(tc.tile_pool(name="weights", bufs=1))
        bigs = ctx.enter_context(tc.tile_pool(name="bigs", bufs=1))
        etp = ctx.enter_context(tc.tile_pool(name="et", bufs=2))
        spool = ctx.enter_context(tc.tile_pool(name="ssum", bufs=2))
        rpool = ctx.enter_context(tc.tile_pool(name="rbc", bufs=1))
        rcpool = ctx.enter_context(tc.tile_pool(name="rcp", bufs=1))
        outp = ctx.enter_context(tc.tile_pool(name="outstage", bufs=2))
        # psum: scores/proj transient tiles (2x 2 banks) + pv accumulators
        # (2 pairs x 2 banks) = 8 banks exactly.
        psq = ctx.enter_context(tc.tile_pool(name="ps_t", bufs=2, space="PSUM"))
        pspv = ctx.enter_context(tc.tile_pool(name="ps_pv", bufs=2, space="PSUM"))

        # ---- persistent sbuf tensors ----
        wqk_sb = wpool.tile([128, KC, 2 * D], BF)
        wv_sb = wpool.tile([128, KC, D], BF)
        wo_sb = wpool.tile([128, KC, D], BF)
        bqk_sb = wpool.tile([128, 2 * KC], FP)
        bo_sb = wpool.tile([128, D], FP)
        xT = bigs.tile([128, KC, S], BF)
        # qkT[:, m, :]: m 0..5 -> qT (heads 2m, 2m+1 on partitions 0:64,
        # 64:128), m 6..11 -> kT likewise.
        qkT = bigs.tile([128, 2 * KC, S], BF)
        vsb = bigs.tile([128, SC, D], BF)       # v in [s-part, sk, h*hd]
        outT = bigs.tile([128, KC, S], BF)

        # ---- input DMAs ----
        # xT per-kc chunks on the scalar queue so the first q/k projections
        # start ~1us in; wqk column-blocks m=0 (q heads 0,1) and m=6
        # (k heads 0,1) first -- they gate pair 0's scores.
        xTr = xT_d.rearrange("(kc p) s -> p kc s", p=128)
        for kc in range(KC):
            nc.scalar.dma_start(xT[:, kc, :], xTr[:, kc, :])
        wqkr = wqk_d.rearrange("(kc p) f -> p kc f", p=128)
        nc.sync.dma_start(wqk_sb[:, :, 0:128], wqkr[:, :, 0:128])
        nc.sync.dma_start(wqk_sb[:, :, D:D + 128], wqkr[:, :, D:D + 128])
        nc.sync.dma_start(bqk_sb[:], bqk_d.rearrange("(j p) -> p j", p=128))
        nc.sync.dma_start(wqk_sb[:, :, 128:D], wqkr[:, :, 128:D])
        nc.sync.dma_start(wqk_sb[:, :, D + 128:], wqkr[:, :, D + 128:])
        nc.sync.dma_start(wv_sb[:],
                          wv_d.rearrange("(kc p) f -> p kc f", p=128))
        nc.sync.dma_start(wo_sb[:],
                          wo_d.rearrange("(kc p) f -> p kc f", p=128))
        nc.sync.dma_start(
            bo_sb[:],
            bo2_d.rearrange("(a f) -> a f", a=1).partition_broadcast(128),
        )

        def qkv_m(m):
            """project one 128-col block of q or k (m 0..5 q, 6..11 k)"""
            ps = psq.tile([128, S], FP, tag="ps", name=f"qk_{m}")
            for kc in range(KC):
                lhsT = wqk_sb[:, kc, m * 128:(m + 1) * 128]
                for n in range(NQ):
                    MM(
                        ps[:, n * 512:(n + 1) * 512],
                        lhsT,
                        xT[:, kc, n * 512:(n + 1) * 512],
                        start=(kc == 0),
                        stop=(kc == KC - 1),
                        reuse_w=(n > 0),
                    )
            nc.vector.tensor_scalar_add(qkT[:, m, :], ps[:], bqk_sb[:, m:m + 1])

        def v_chunk(sc):
            ps = psq.tile([128, S], FP, tag="ps", name=f"v_{sc}")
            for kc in range(KC):
                lhsT = xT[:, kc, sc * 128:(sc + 1) * 128]
                MM(ps[:, 0:512], lhsT, wv_sb[:, kc, 0:512],
                   start=(kc == 0), stop=(kc == KC - 1))
                MM(ps[:, 512:D], lhsT, wv_sb[:, kc, 512:D],
                   start=(kc == 0), stop=(kc == KC - 1), reuse_w=True)
            nc.vector.tensor_copy(vsb[:, sc, :], ps[:, 0:D])

        def scores_chunk(t, sk, et_t, s_t):
            for h01 in range(2):
                ps = psq.tile([128, S], FP, tag="ps", name=f"sc_{t}_{sk}_{h01}")
                lo, hi = h01 * 64, (h01 + 1) * 64
                lhsT = qkT[lo:hi, KC + t, sk * 128:(sk + 1) * 128]
                for n in range(NQ):
                    MM(
                        ps[:, n * 512:(n + 1) * 512],
                        lhsT,
                        qkT[lo:hi, t, n * 512:(n + 1) * 512],
                        start=True,
                        stop=True,
                        tile_position=(h01 * 64, 0),
                        reuse_w=(n > 0),
                    )
                nc.scalar.activation(
                    et_t[:, h01, sk, :], ps[:], ActFn.Exp, scale=SCALE
                )
            # running denominator: S[:, h01, :] += et chunk (bf16, 2 heads in
            # one strided op)
            if sk == 0:
                nc.vector.tensor_copy(s_t[:], et_t[:, :, 0, :])
            else:
                nc.vector.tensor_tensor(s_t[:], s_t[:], et_t[:, :, sk, :],
                                        op=AluOp.add)

        def pv_chunk(t, sk, et_t, pv_ps):
            # two heads as concurrent col-tiled matmuls: head a -> array
            # cols / psum partitions 0:64, head b -> 64:128 (tile_position
            # auto-derives from the psum slice base partition)
            for n in range(NQ):
                for h01 in range(2):
                    h = 2 * t + h01
                    MM(
                        pv_ps[h01 * 64:(h01 + 1) * 64, n * 512:(n + 1) * 512],
                        vsb[:, sk, h * 64:(h + 1) * 64],
                        et_t[:, h01, sk, n * 512:(n + 1) * 512],
                        start=(sk == 0),
                        stop=(sk == SC - 1),
                        skip_group_check=True,
                    )

        def pv_finalize(t, s_t, pv_ps):
            # r (replicated to all partitions) via gpsimd partition-sum of
            # the accumulated exp sums; fast Newton reciprocal; then divide
            # straight out of the PV psum accumulator into outT.
            rbc = rpool.tile([128, 2, S], FP, tag="rbc", name=f"rbc_{t}")
            nc.gpsimd.partition_all_reduce(rbc[:], s_t[:], 128, ReduceOp.add)
            rcp = rcpool.tile([128, 2, S], FP, tag="rcp", name=f"rcp_{t}")
            nc.vector.reciprocal_approx_fast(rcp[:], rbc[:])
            for h01 in range(2):
                lo, hi = h01 * 64, (h01 + 1) * 64
                nc.vector.tensor_tensor(
                    outT[lo:hi, t, :],
                    pv_ps[lo:hi, :],
                    rcp[lo:hi, h01, :],
                    op=AluOp.mult,
                )

        # ---- main pipeline ----
        # Flat software pipeline over 48 (pair, sk) chunks: pv(j-2) rides 2
        # chunk-slots behind scores(j), crossing pair boundaries, so neither
        # TensorE nor ScalarE ever drains.
        et_tiles = {}
        s_tiles = {}
        pv_tiles = {}

        def emit_pv(j):
            t, sk = j // SC, j % SC
            if sk == 0:
                pv_tiles[t] = pspv.tile([128, S], FP, tag="pv",
                                        name=f"pv_{t}")
            pv_chunk(t, sk, et_tiles[t], pv_tiles[t])
            if sk == SC - 1:
                pv_finalize(t, s_tiles[t], pv_tiles[t])
                del pv_tiles[t], et_tiles[t], s_tiles[t]

        # pair 0's q/k projections gate the whole pipeline
        qkv_m(0)
        qkv_m(KC)

        NCH = NP * SC
        for j in range(NCH):
            t, sk = j // SC, j % SC
            if sk == 0:
                et_tiles[t] = etp.tile([128, 2, SC, S], BF, tag="et",
                                       name=f"et_{t}")
                s_tiles[t] = spool.tile([128, 2, S], BF, tag="s",
                                        name=f"s_{t}")
            scores_chunk(t, sk, et_tiles[t], s_tiles[t])
            if t == 0:
                v_chunk(sk)
            if t + 1 < NP:
                if sk == 3:
                    qkv_m(t + 1)
                elif sk == 4:
                    qkv_m(KC + t + 1)
            if j >= 2:
                emit_pv(j - 2)
        emit_pv(NCH - 2)
        emit_pv(NCH - 1)

        # ---- output projection ----
        for sc in range(SC):
            ps = psq.tile([128, S], FP, tag="ps", name=f"o_{sc}")
            for kc in range(KC):
                lhsT = outT[:, kc, sc * 128:(sc + 1) * 128]
                MM(ps[:, 0:512], lhsT, wo_sb[:, kc, 0:512],
                   start=(kc == 0), stop=(kc == KC - 1))
                MM(ps[:, 512:D], lhsT, wo_sb[:, kc, 512:D],
                   start=(kc == 0), stop=(kc == KC - 1), reuse_w=True)
            osb = outp.tile([128, D], BF, tag="osb", name=f"osb_{sc}")
            nc.vector.tensor_tensor(osb[:], ps[:, 0:D], bo_sb[:], op=AluOp.add)
            (nc.scalar if sc % 2 else nc.sync).dma_start(
                out_d[sc * 128:(sc + 1) * 128, :], osb[:])


def build():
    """Build + compile the per-core Bass module. Returns the Bacc object."""
    nc = bacc.Bacc("TRN2", target_bir_lowering=False, debug=False, num_devices=B)
    xT_d = nc.dram_tensor("xT", [D, S], BF, kind="ExternalInput").ap()
    wqk_d = nc.dram_tensor("wqk", [D, 2 * D], BF, kind="ExternalInput").ap()
    wv_d = nc.dram_tensor("wv", [D, D], BF, kind="ExternalInput").ap()
    wo_d = nc.dram_tensor("wo", [D, D], BF, kind="ExternalInput").ap()
    bqk_d = nc.dram_tensor("bqk", [2 * D], FP, kind="ExternalInput").ap()
    bo2_d = nc.dram_tensor("bo2", [D], FP, kind="ExternalInput").ap()
    out_d = nc.dram_tensor("out", [S, D], BF, kind="ExternalOutput").ap()
    with tile.TileContext(nc) as tc:
        _build_kernel_body(tc, out_d, xT_d, wqk_d, wv_d, wo_d, bqk_d, bo2_d)
    nc.compile()
    return nc


def prep_weights(Wqkv, bqkv, Wo, bo):
    """Host-side weight packing (numpy only)."""
    # Wqkv [H, D, 3*HD] -> Wq_all/Wk_all/Wv_all [D, H*HD]
    Wq = np.transpose(Wqkv[:, :, 0:HD], (1, 0, 2)).reshape(D, D)
    Wk = np.transpose(Wqkv[:, :, HD:2 * HD], (1, 0, 2)).reshape(D, D)
    Wv = np.transpose(Wqkv[:, :, 2 * HD:], (1, 0, 2)).reshape(D, D)
    wqk = np.concatenate([Wq, Wk], axis=1)  # [D, 2D]
    bq = bqkv[:, 0:HD].reshape(D)
    bk = bqkv[:, HD:2 * HD].reshape(D)
    bv = bqkv[:, 2 * HD:].reshape(D)
    bqk = np.concatenate([bq, bk])  # [2D]
    bo2 = bo.astype(np.float64) + bv.astype(np.float64) @ Wo.astype(np.float64)
    bf16 = ml_dtypes.bfloat16
    return {
        "wqk": np.ascontiguousarray(wqk.astype(bf16)),
        "wv": np.ascontiguousarray(Wv.astype(bf16)),
        "wo": np.ascontiguousarray(Wo.astype(bf16)),
        "bqk": np.ascontiguousarray(bqk.astype(np.float32)),
        "bo2": np.ascontiguousarray(bo2.astype(np.float32)),
    }


def prep_core_inputs(x, Wqkv, bqkv, Wo, bo):
    """Full host-side preprocessing -> list of per-core input maps."""
    w = prep_weights(np.asarray(Wqkv), np.asarray(bqkv), np.asarray(Wo),
                     np.asarray(bo))
    x = np.asarray(x, dtype=np.float32)
    bf16 = ml_dtypes.bfloat16
    return [
        {"xT": np.ascontiguousarray(x[i].T.astype(bf16)), **w}
        for i in range(B)
    ]


_nc_cache = None


def kernel(x, Wqkv, bqkv, Wo, bo):
    global _nc_cache, last_results
    if _nc_cache is None:
        _nc_cache = build()
    nc = _nc_cache
    in_maps = prep_core_inputs(x, Wqkv, bqkv, Wo, bo)
    res = run_bass_kernel_spmd(
        nc, in_maps, core_ids=list(range(B)),
        trace=bool(os.environ.get("KERNEL_TRACE")),
    )
    last_results = res
    out = np.stack([res.results[i]["out"] for i in range(B)], axis=0)
    return out.astype(np.float32)
